# revision 47
# baseline (speedup 1.0000x reference)
"""MoE layer (T=4096, H=1024, F=2048, E=8, top-2) on 8 Trainium2 NeuronCores.

Strategy (expert-parallel, per the sharding hint):
  - Router runs on host (67 MFLOP, 0.02% of total work) to produce the
    token->expert dispatch; this implements the "all-to-all dispatch by
    routed expert" as host-side sharding, which is where sharding lives in
    this harness's contract (full inputs in, full output out).
  - Core e holds expert e's weights (w1[e], w2[e]) and processes only the
    tokens routed to it (capacity-padded to the max expert load).
  - Device per core: hT = w1[e]^T-oriented matmul producing g^T = gelu(x@w1)
    in [F, C] layout, then y^T = w2^T-stationary matmuls over g^T -- no
    on-device transposes needed; prob scaling happens in the host combine.
  - Host scatter-adds each expert's [count, H] slice into the [T, H] output.

Matmuls run in bf16 with fp32 PSUM accumulation (rel err ~3e-3 vs the fp32
reference on this input set; measured host-side before committing to it).
"""

import numpy as np
import ml_dtypes

T, H, F, E, TOPK = 4096, 1024, 2048, 8, 2

_BUILT = {}  # cache: (C, maxc, dtype_tag, gelu_tag) -> bass.Bass


def _chunks_a(C):
    """Phase A chunk layout. Smaller head chunk than phase B: with the paced
    w1 stream split across BOTH rings (~250 GB/s), a 384-wide chunk0 pass
    (~20.5us) still covers the 3.25 MB w1 load, and the smaller xT-chunk0
    prefix lets the first real matmul start ~1us earlier. All chunks >= ~130
    so LDWEIGHTS stays hidden."""
    if C <= 640:
        return [(0, C)]
    if C <= 1024:
        return [(0, 512), (512, C - 512)]
    r1 = (C + 2) // 3 // 2 * 2
    r2 = (C - r1 + 1) // 2 // 2 * 2
    return [(0, r1), (r1, r2), (r1 + r2, C - r1 - r2)]


def _route(hidden_states, router_weight):
    """Host router: returns (token_idx[e], prob[e]) per expert."""
    logits = hidden_states.astype(np.float32) @ router_weight.astype(np.float32)
    # top-2 by value (descending); ties broken like jax.lax.top_k (lowest index)
    order = np.argsort(-logits, axis=1, kind="stable")
    top_idx = order[:, :TOPK]                                   # [T, 2]
    top_vals = np.take_along_axis(logits, top_idx, axis=1)      # [T, 2]
    m = top_vals.max(axis=1, keepdims=True)
    ex = np.exp(top_vals - m)
    top_probs = ex / ex.sum(axis=1, keepdims=True)              # [T, 2]
    tok_per_e, prob_per_e = [], []
    for e in range(E):
        mask = top_idx == e                                     # [T, 2]
        tok = np.nonzero(mask.any(axis=1))[0]
        p = (top_probs * mask).sum(axis=1)[tok].astype(np.float32)
        tok_per_e.append(tok)
        prob_per_e.append(p)
    return tok_per_e, prob_per_e


def _strip_redundant_same_engine_waits(nc, margin=8):
    """Drop sem waits that FIFO program order on the same engine already
    guarantees (cumulative same-engine updates >= wait_value + margin).

    Tile occasionally synthesizes such waits (e.g. on the first phase-B copy);
    they are semantically redundant but overflow the 1-wait sync budget of the
    AC/TS instruction structs in walrus codegen. margin covers the engine
    queue depth so even in-flight-but-unretired updates can't be relied on.
    """
    from collections import defaultdict
    for f in nc.m.functions:
        counts = defaultdict(int)  # sem_id -> updates so far (program order)
        for b in f.blocks:
            for i in b.instructions:
                si = i.sync_info
                if si is None:
                    continue
                ups = [u for u in (si.on_update or [])
                       if u.sync_type == "semaphore"
                       and u.update_mode in ("sem-inc", "sem-add-imm")]
                own_sems = {u.id for u in ups}
                ws = list(si.on_wait or [])
                keep = []
                for w in ws:
                    if (w.sync_type == "semaphore"
                            and w.wait_mode == "sem-ge-imm"
                            and w.id in own_sems):
                        # wait on the stream this instruction itself belongs
                        # to (it updates the same sem): FIFO order within the
                        # stream makes it redundant once enough prior updates
                        # exist. DMA queue sems bump by 16 per transfer, so
                        # margin only applies to 1-inc engine sems.
                        m = margin if all(
                            u.update_value == 1 for u in ups if u.id == w.id
                        ) else 0
                        if counts[w.id] >= w.wait_value + m:
                            continue
                    keep.append(w)
                if len(keep) != len(ws):
                    si.on_wait = keep
                    i.sync_info = si
                for u in ups:
                    counts[u.id] += u.update_value


def _split_overloaded_waits(nc, max_waits=1):
    """The TPB instruction structs encode at most one sem-ge wait (plus
    updates); walrus errors with "Too many sync wait commands" beyond that.
    For any instruction still carrying several sem-ge waits after the
    redundancy strip, move the excess onto NOPs inserted just before it on
    the same engine -- a strictly more conservative ordering."""
    import concourse.mybir as mybir
    import bass_rust
    for f in nc.m.functions:
        for b in f.blocks:
            il = b.instructions
            idx = 0
            while idx < len(il):
                i = il[idx]
                si = i.sync_info
                if type(i).__name__ == "InstEventSemaphore" or si is None:
                    idx += 1
                    continue
                sem_ws = [w for w in (si.on_wait or [])
                          if w.sync_type == "semaphore"
                          and w.wait_mode == "sem-ge-imm"]
                other_ws = [w for w in (si.on_wait or []) if w not in sem_ws]
                if len(sem_ws) > max_waits:
                    si.on_wait = other_ws + sem_ws[-max_waits:]
                    i.sync_info = si
                    rest = sem_ws[:-max_waits]
                    pos = idx
                    for j in range(0, len(rest), max_waits):
                        n = mybir.InstNoOp(
                            name=nc.get_next_instruction_name(),
                            ins=[], outs=[])
                        n.engine = i.engine
                        n.sync_info = bass_rust.SyncInfo(
                            on_wait=rest[j:j + max_waits], on_update=[])
                        il.insert(pos, n)
                        pos += 1
                        idx += 1
                idx += 1


def _trim_redundant_barriers(nc):
    """Remove the Tile entry barrier from the main block (delays the first
    DMA trigger ~0.7us; body data deps are fully covered by per-tile sems --
    the only thing it orders is const-tile memsets, which finish ~8us before
    their first reader) and the second end-block barrier round (the runtime
    epilogue's own butterfly follows immediately)."""
    bar_ids = {int(k) for k, v in nc.m.ant_sem_names.items()
               if any("barrier_" in n for n in v)}

    def is_bar(i):
        si = i.sync_info
        if si is None or type(i).__name__ not in ("InstDrain",
                                                  "InstEventSemaphore"):
            return False
        ids = [w.id for w in (si.on_wait or [])] +               [u.id for u in (si.on_update or [])]
        return bool(ids) and all(x in bar_ids for x in ids)

    f = nc.m.functions[0]
    main = f.blocks[0]
    main.instructions[:] = [i for i in main.instructions if not is_bar(i)]
    endb = f.blocks[-1]
    il = endb.instructions
    isa = [ix for ix, i in enumerate(il) if type(i).__name__ == "InstISA"]
    if isa:
        cut = isa[-1] + 1
        il[:] = il[:cut] + [i for i in il[cut:] if not is_bar(i)]


def _build(C, dtype_tag="bf16", gelu_tag="tanh", valid_c=None):
    """Build the per-core Bass program. C = token capacity (multiple of 4,
    set to the exact max expert load rounded up); every column is computed."""
    import concourse.bass as bass
    import concourse.mybir as mybir
    import concourse.tile as tile

    DT = {"bf16": mybir.dt.bfloat16, "f32": mybir.dt.float32}[dtype_tag]
    F32 = mybir.dt.float32
    GELU = {
        "tanh": mybir.ActivationFunctionType.Gelu_apprx_tanh,
        "erf": mybir.ActivationFunctionType.Gelu,
    }[gelu_tag]
    mm_cast = (lambda ap: ap.bitcast(mybir.dt.float32r)) if dtype_tag == "f32" \
        else (lambda ap: ap)

    KH, KF = H // 128, F // 128        # 8 h-tiles, 16 f-tiles
    # Chunk layout: a 512-wide head (paces w1 streaming at ~150 GB/s during
    # the first pass) + the remainder split evenly. Every chunk must be
    # >= ~130 columns: narrower matmuls cannot hide their 128-col LDWEIGHTS
    # (~53 ns with FWL), which floors the cadence regardless of width.
    # Phase A chunks: 512-wide head (so the chunk0 pass lasts >= the ~23us
    # the w1 stream needs at ring bandwidth) + even remainder, all >= ~130
    # columns so LDWEIGHTS (~53ns/tile, unhideable below 128 cols) stays
    # covered by the matmul stream.
    # Phase B chunks: big-to-small with a 128-wide tail -- a 128-wide matmul
    # exactly matches the LDWEIGHTS floor (zero waste) and makes the final
    # copy+DMA drain tiny. gT lives in SBUF, so B may chunk independently.
    chunks_a = _chunks_a(C)
    if C <= 640:
        chunks_b = chunks_a
    elif C <= 1024:
        chunks_b = [(0, 512), (512, C - 512)]
    else:
        chunks_b = [(0, 512), (512, C - 640), (C - 128, 128)]
    chunks = chunks_a

    nc = bass.Bass()
    # x arrives pre-transposed (host transpose is free; transpose-DMA from
    # token-major was ~2x slower AND corrupted repeat executions). It is
    # split into a chunk0 tensor and a rest tensor so every DMA slice is
    # FULLY contiguous in DRAM (row stride == row length) -- lets the DMA
    # concatenate M2S reads instead of 1 KB-per-descriptor fetches on the
    # head's critical path.
    cw0 = chunks[0][1]
    xT0_d = nc.dram_tensor("xT0", [H, cw0], DT, kind="ExternalInput")
    xTr_d = nc.dram_tensor("xTr", [H, C - cw0], DT, kind="ExternalInput") \
        if C > cw0 else None
    # w1 arrives pre-tiled: w1t[fm, p, hk*128+c] = w1[hk*128+p, fm*128+c], so
    # each [128, 128] lhsT tile is a contiguous free-dim slice and one fm's
    # worth (0.5 MB) is a single DMA -- keeps the first matmul group's DMA
    # prefix tiny instead of needing all of w1.
    w1_d = nc.dram_tensor("w1t", [KF, 128, KH * 128], DT, kind="ExternalInput")
    w2_d = nc.dram_tensor("w2", [F, H], DT, kind="ExternalInput")
    # y is produced transposed ([H, C]): phase B keeps tokens on the moving
    # axis. bf16 output halves the out-DMA bytes; the host combines in fp32.
    y_d = nc.dram_tensor("y", [H, C], DT, kind="ExternalOutput")

    with tile.TileContext(nc) as tc:
        with tc.tile_pool(name="persist", bufs=1) as wp, \
             tc.tile_pool(name="psumA", bufs=5, space="PSUM") as pp_a, \
             tc.tile_pool(name="psumB", bufs=3, space="PSUM") as pp_b, \
             tc.tile_pool(name="outs", bufs=4) as op:

            xT_s = [wp.tile([128, C], DT, name=f"xT{k}") for k in range(KH)]
            w1_s = [wp.tile([128, KH * 128], DT, name=f"w1t{k}")
                    for k in range(KF)]
            w2_s = [wp.tile([128, H], DT, name=f"w2{k}") for k in range(KF)]

            # DMA issue order tracks first use. Phase A runs chunk-outer, so
            # the critical prefix is w1[0] (256 KB, the first group's weights)
            # + xT chunk0 (1 MB); later groups only need one 256 KB w1 tile
            # each ~2us. Only SP + ACT can push HWDGE.
            def xchunk(k, c0, cw, e):
                if c0 < cw0:
                    src = xT0_d[k * 128:(k + 1) * 128, c0:c0 + cw]
                else:
                    src = xTr_d[k * 128:(k + 1) * 128, c0 - cw0:c0 - cw0 + cw]
                e.dma_start(xT_s[k][:, c0:c0 + cw], src)

            # Critical-prefix DMAs only: xT chunk0 + w1[0..2]. All other
            # input DMAs are paced behind phase-A progress (below).
            # w1[0] gates the very first matmul; split it across the two
            # rings BY PARTITION so each half keeps full 2 KB descriptors.
            (c0a, cwa) = chunks[0]
            # Byte-balanced rings: scalar carries w1[0]h1 + w1[1] + xT0-3,
            # sync carries w1[0]h2 + xT4-7 + w1[2] (~750 KB each). w1[1]
            # rides between the first xT tiles so group 1 never waits on it.
            # Offloading w1[1/2] to the GpSimd SWDGE ring: +3.5us, do not
            # resurrect.
            nc.scalar.dma_start(w1_s[0][0:64, :], w1_d[0][0:64, :])
            nc.sync.dma_start(w1_s[0][64:128, :], w1_d[0][64:128, :])
            xchunk(0, c0a, cwa, nc.scalar)
            nc.scalar.dma_start(w1_s[1][:], w1_d[1])
            xchunk(1, c0a, cwa, nc.scalar)
            xchunk(2, c0a, cwa, nc.scalar)
            xchunk(3, c0a, cwa, nc.scalar)
            for k in range(4, KH):
                xchunk(k, c0a, cwa, nc.sync)
            nc.sync.dma_start(w1_s[2][:], w1_d[2])

            # PE warm-up: the critical prefix (w1[0] + xT chunk0, ~1.25 MB)
            # cannot land before ~5us after the rings start, so the first
            # real matmul is DMA-gated no matter what. Dummy matmuls soak
            # that window: they keep HAM's activity window busy so the clock
            # flips 1.2->2.4 GHz before the real stream begins (real work
            # then runs entirely warm). 12 x 427ns cold covers the prefix.
            dmy_l = wp.tile([128, 128], DT, name="dmy_l")
            dmy_r = wp.tile([128, 512], DT, name="dmy_r")
            nc.vector.memset(dmy_l[:], 0)
            nc.vector.memset(dmy_r[:], 0)
            ps_w = pp_b.tile([128, 512], F32, name="psW", tag="psB")
            for r in range(11):
                nc.tensor.matmul(ps_w[:], mm_cast(dmy_l[:]), mm_cast(dmy_r[:]),
                                 start=(r == 0), stop=(r == 10))
            gT_s = [wp.tile([128, C], DT, name=f"gT{k}") for k in range(KF)]

            # Non-critical input DMAs, paced behind phase-A groups so the
            # transfers trail compute instead of competing with the critical
            # prefix for aggregate HBM bandwidth. pace_plan[g] = DMAs to
            # issue once group g's matmuls retire (~1.7us per warm group).
            pace_plan = {}
            pace_flip = [0]

            def plan(g, dst, src, tr=False):
                pace_plan.setdefault(g, []).append((dst, src, tr))

            for k in range(3, KF):
                plan(max(0, k - 6), w1_s[k][:], w1_d[k])   # needed at group k
            for ci, (c0, cw) in enumerate(chunks[1:]):     # needed at chunk 1+
                for k in range(KH):
                    plan(4 + 5 * ci + k // 2, xT_s[k][:, c0:c0 + cw],
                         xTr_d[k * 128:(k + 1) * 128, c0 - cw0:c0 - cw0 + cw])
            # w2 is only needed at phase B, and phase B's FIRST group already
            # contracts over all 16 fk tiles -- so w2 must finish by phase A
            # end but should stay off the rings during the chunk0 pass, where
            # w1 streaming already saturates them (~150 GB/s consumption).
            for k in range(KF):
                plan(14 + 2 * k, w2_s[k][:], w2_d[k * 128:(k + 1) * 128, :])

            from concourse.tile_rust import add_dep_helper

            # Phase A: g^T[f, c] = gelu(sum_h w1[h, f] * x[c, h]), [F, C]
            # layout. chunk-outer: early groups reuse xT chunk0 and need just
            # one fresh w1 tile each, keeping the DMA critical path short.
            gi = 0
            for (c0, cw) in chunks:
                for fm in range(KF):
                    ps = pp_a.tile([128, 512], F32, name="psA", tag="psA")
                    for hk in range(KH):
                        mm = nc.tensor.matmul(
                            ps[:, :cw],
                            mm_cast(w1_s[fm][:, hk * 128:(hk + 1) * 128]),
                            mm_cast(xT_s[hk][:, c0:c0 + cw]),
                            start=(hk == 0),
                            stop=(hk == KH - 1),
                        )
                    nc.scalar.activation(gT_s[fm][:, c0:c0 + cw], ps[:, :cw], GELU)
                    # Paced transfers alternate rings: a single ring caps the
                    # w1 stream at ~130 GB/s, which a 364-wide chunk0 pass
                    # would outrun.
                    for (dst, src, tr) in pace_plan.get(gi, []):
                        eng = nc.sync if pace_flip[0] else nc.scalar
                        pace_flip[0] ^= 1
                        d = eng.dma_start(dst, src, transpose=tr)
                        add_dep_helper(d.ins, mm.ins, sync=True,
                                       reason="dma pacing")
                    gi += 1

            # Scheduler fence between phases: without it Tile reorders the ACT
            # stream and synthesizes same-engine waits on the phase-B copies,
            # overflowing the AC struct's 1-wait budget.
            tc.no_sync_barrier()

            # Phase B: y^T[h, c] = sum_f w2[f, h] * g^T[f, c].
            # chunk-outer with the narrow chunk LAST: out-DMAs trickle behind
            # compute and the final drain is only the narrow chunk's 8 small
            # tiles instead of a full 1.2 MB hm-row. Routing-prob scaling
            # happens on the host during scatter-add.
            for ci, (c0, cw) in enumerate(chunks_b):
                last_chunk = ci == len(chunks_b) - 1
                for hm in range(KH):
                    ps = pp_b.tile([128, 512], F32, name="psB", tag="psB")
                    for fk in range(KF):
                        nc.tensor.matmul(
                            ps[:, :cw],
                            mm_cast(w2_s[fk][:, hm * 128:(hm + 1) * 128]),
                            mm_cast(gT_s[fk][:, c0:c0 + cw]),
                            start=(fk == 0),
                            stop=(fk == KF - 1),
                        )
                    # unique slot per output tile: a reused slot would add a
                    # WAR wait on the DMA-out to the copy instruction.
                    yt = op.tile([128, 512], DT, name="yt", tag="yt",
                                 bufs=KH * len(chunks_b))
                    nc.scalar.activation(
                        yt[:, :cw], ps[:, :cw],
                        mybir.ActivationFunctionType.Copy)
                    # Triggers on SP (idle in phase B; a trigger costs ~620ns
                    # of issuing-engine time). Last chunk: tiles complete
                    # every ~460ns, faster than one ring can trigger, so
                    # alternate with ACT to keep the final DMA tight behind
                    # its copy.
                    deng = (nc.scalar if hm % 2 else nc.sync) if last_chunk \
                        else nc.sync
                    deng.dma_start(
                        y_d[hm * 128:(hm + 1) * 128, c0:c0 + cw],
                        yt[:, :cw])

    _strip_redundant_same_engine_waits(nc)
    _split_overloaded_waits(nc)
    _trim_redundant_barriers(nc)
    return nc


def _make_in_maps(hidden_states, w1, w2, tok_per_e, prob_per_e, C,
                  dtype_tag="bf16"):
    np_dt = {"bf16": ml_dtypes.bfloat16, "f32": np.float32}[dtype_tag]
    cw0 = _chunks_a(C)[0][1]
    in_maps = []
    for e in range(E):
        tok = tok_per_e[e]
        xg = np.zeros((H, C), dtype=np_dt)
        xg[:, :len(tok)] = hidden_states[tok].T.astype(np_dt)
        KH, KF = H // 128, F // 128
        w1t = (w1[e].astype(np_dt).reshape(KH, 128, KF, 128)
               .transpose(2, 1, 0, 3).reshape(KF, 128, KH * 128))
        m = {
            "xT0": np.ascontiguousarray(xg[:, :cw0]),
            "w1t": np.ascontiguousarray(w1t),
            "w2": np.ascontiguousarray(w2[e].astype(np_dt)),
        }
        if C > cw0:
            m["xTr"] = np.ascontiguousarray(xg[:, cw0:])
        in_maps.append(m)
    return in_maps


def kernel(hidden_states, router_weight, w1, w2):
    from concourse.bass_utils import run_bass_kernel_spmd

    hidden_states = np.asarray(hidden_states, dtype=np.float32)
    router_weight = np.asarray(router_weight, dtype=np.float32)
    w1 = np.asarray(w1, dtype=np.float32)
    w2 = np.asarray(w2, dtype=np.float32)

    tok_per_e, prob_per_e = _route(hidden_states, router_weight)
    maxc = max(len(t) for t in tok_per_e)
    C = max(128, -(-maxc // 4) * 4)              # exact capacity, mult of 4

    dtype_tag, gelu_tag = "bf16", "tanh"
    key = (C, maxc, dtype_tag, gelu_tag)
    if key not in _BUILT:
        _BUILT[key] = _build(C, dtype_tag, gelu_tag, valid_c=maxc)
    nc = _BUILT[key]

    in_maps = _make_in_maps(hidden_states, w1, w2, tok_per_e, prob_per_e, C,
                            dtype_tag)

    res = run_bass_kernel_spmd(nc, in_maps, core_ids=list(range(E)))

    out = np.zeros((T, H), dtype=np.float32)
    for e in range(E):
        tok = tok_per_e[e]
        yT = res.results[e]["y"]          # [H, C] bf16, pad columns junk
        out[tok] += prob_per_e[e][:, None] * \
            yT[:, :len(tok)].T.astype(np.float32)
    return out



# revision 53
# speedup vs baseline: 1.0120x; 1.0120x over previous
"""MoE layer (T=4096, H=1024, F=2048, E=8, top-2) on 8 Trainium2 NeuronCores.

Strategy (expert-parallel, per the sharding hint):
  - Router runs on host (67 MFLOP, 0.02% of total work) to produce the
    token->expert dispatch; this implements the "all-to-all dispatch by
    routed expert" as host-side sharding, which is where sharding lives in
    this harness's contract (full inputs in, full output out).
  - Core e holds expert e's weights (w1[e], w2[e]) and processes only the
    tokens routed to it (capacity-padded to the max expert load).
  - Device per core: hT = w1[e]^T-oriented matmul producing g^T = gelu(x@w1)
    in [F, C] layout, then y^T = w2^T-stationary matmuls over g^T -- no
    on-device transposes needed; prob scaling happens in the host combine.
  - Host scatter-adds each expert's [count, H] slice into the [T, H] output.

Matmuls run in bf16 with fp32 PSUM accumulation (rel err ~3e-3 vs the fp32
reference on this input set; measured host-side before committing to it).
"""

import numpy as np
import ml_dtypes

T, H, F, E, TOPK = 4096, 1024, 2048, 8, 2

_BUILT = {}  # cache: (C, maxc, dtype_tag, gelu_tag) -> bass.Bass


def _chunks_a(C):
    """Phase A chunk layout: 512-wide head + even remainder, all >= ~130.
    chunk0 MUST stay 512: the w1 stream is feedback-paced off matmul-group
    retirement and tops out ~150 GB/s, which exactly matches a 512-wide
    chunk0 pass's consumption. A 364-wide chunk0 was tried (smaller prefix,
    2-ring pacing): the pass outran the w1 supply, +2.6us of stalls."""
    if C <= 640:
        return [(0, C)]
    if C <= 1024:
        return [(0, 512), (512, C - 512)]
    r = C - 512
    r1 = (r + 3) // 4 * 2
    return [(0, 512), (512, r1), (512 + r1, r - r1)]


def _route(hidden_states, router_weight):
    """Host router: returns (token_idx[e], prob[e]) per expert."""
    logits = hidden_states.astype(np.float32) @ router_weight.astype(np.float32)
    # top-2 by value (descending); ties broken like jax.lax.top_k (lowest index)
    order = np.argsort(-logits, axis=1, kind="stable")
    top_idx = order[:, :TOPK]                                   # [T, 2]
    top_vals = np.take_along_axis(logits, top_idx, axis=1)      # [T, 2]
    m = top_vals.max(axis=1, keepdims=True)
    ex = np.exp(top_vals - m)
    top_probs = ex / ex.sum(axis=1, keepdims=True)              # [T, 2]
    tok_per_e, prob_per_e = [], []
    for e in range(E):
        mask = top_idx == e                                     # [T, 2]
        tok = np.nonzero(mask.any(axis=1))[0]
        p = (top_probs * mask).sum(axis=1)[tok].astype(np.float32)
        tok_per_e.append(tok)
        prob_per_e.append(p)
    return tok_per_e, prob_per_e


def _strip_redundant_same_engine_waits(nc, margin=8):
    """Drop sem waits that FIFO program order on the same engine already
    guarantees (cumulative same-engine updates >= wait_value + margin).

    Tile occasionally synthesizes such waits (e.g. on the first phase-B copy);
    they are semantically redundant but overflow the 1-wait sync budget of the
    AC/TS instruction structs in walrus codegen. margin covers the engine
    queue depth so even in-flight-but-unretired updates can't be relied on.
    """
    from collections import defaultdict
    for f in nc.m.functions:
        counts = defaultdict(int)  # sem_id -> updates so far (program order)
        for b in f.blocks:
            for i in b.instructions:
                si = i.sync_info
                if si is None:
                    continue
                ups = [u for u in (si.on_update or [])
                       if u.sync_type == "semaphore"
                       and u.update_mode in ("sem-inc", "sem-add-imm")]
                own_sems = {u.id for u in ups}
                ws = list(si.on_wait or [])
                keep = []
                for w in ws:
                    if (w.sync_type == "semaphore"
                            and w.wait_mode == "sem-ge-imm"
                            and w.id in own_sems):
                        # wait on the stream this instruction itself belongs
                        # to (it updates the same sem): FIFO order within the
                        # stream makes it redundant once enough prior updates
                        # exist. DMA queue sems bump by 16 per transfer, so
                        # margin only applies to 1-inc engine sems.
                        m = margin if all(
                            u.update_value == 1 for u in ups if u.id == w.id
                        ) else 0
                        if counts[w.id] >= w.wait_value + m:
                            continue
                    keep.append(w)
                if len(keep) != len(ws):
                    si.on_wait = keep
                    i.sync_info = si
                for u in ups:
                    counts[u.id] += u.update_value


def _split_overloaded_waits(nc, max_waits=1):
    """The TPB instruction structs encode at most one sem-ge wait (plus
    updates); walrus errors with "Too many sync wait commands" beyond that.
    For any instruction still carrying several sem-ge waits after the
    redundancy strip, move the excess onto NOPs inserted just before it on
    the same engine -- a strictly more conservative ordering."""
    import concourse.mybir as mybir
    import bass_rust
    for f in nc.m.functions:
        for b in f.blocks:
            il = b.instructions
            idx = 0
            while idx < len(il):
                i = il[idx]
                si = i.sync_info
                if type(i).__name__ == "InstEventSemaphore" or si is None:
                    idx += 1
                    continue
                sem_ws = [w for w in (si.on_wait or [])
                          if w.sync_type == "semaphore"
                          and w.wait_mode == "sem-ge-imm"]
                other_ws = [w for w in (si.on_wait or []) if w not in sem_ws]
                if len(sem_ws) > max_waits:
                    si.on_wait = other_ws + sem_ws[-max_waits:]
                    i.sync_info = si
                    rest = sem_ws[:-max_waits]
                    pos = idx
                    for j in range(0, len(rest), max_waits):
                        n = mybir.InstNoOp(
                            name=nc.get_next_instruction_name(),
                            ins=[], outs=[])
                        n.engine = i.engine
                        n.sync_info = bass_rust.SyncInfo(
                            on_wait=rest[j:j + max_waits], on_update=[])
                        il.insert(pos, n)
                        pos += 1
                        idx += 1
                idx += 1


def _trim_redundant_barriers(nc):
    """Remove the Tile entry barrier from the main block (delays the first
    DMA trigger ~0.7us; body data deps are fully covered by per-tile sems --
    the only thing it orders is const-tile memsets, which finish ~8us before
    their first reader) and the second end-block barrier round (the runtime
    epilogue's own butterfly follows immediately)."""
    bar_ids = {int(k) for k, v in nc.m.ant_sem_names.items()
               if any("barrier_" in n for n in v)}

    def is_bar(i):
        si = i.sync_info
        if si is None or type(i).__name__ not in ("InstDrain",
                                                  "InstEventSemaphore"):
            return False
        ids = [w.id for w in (si.on_wait or [])] +               [u.id for u in (si.on_update or [])]
        return bool(ids) and all(x in bar_ids for x in ids)

    f = nc.m.functions[0]
    main = f.blocks[0]
    main.instructions[:] = [i for i in main.instructions if not is_bar(i)]
    endb = f.blocks[-1]
    il = endb.instructions
    isa = [ix for ix, i in enumerate(il) if type(i).__name__ == "InstISA"]
    if isa:
        cut = isa[-1] + 1
        il[:] = il[:cut] + [i for i in il[cut:] if not is_bar(i)]


def _build(C, dtype_tag="bf16", gelu_tag="tanh", valid_c=None):
    """Build the per-core Bass program. C = token capacity (multiple of 4,
    set to the exact max expert load rounded up); every column is computed."""
    import concourse.bass as bass
    import concourse.mybir as mybir
    import concourse.tile as tile

    DT = {"bf16": mybir.dt.bfloat16, "f32": mybir.dt.float32}[dtype_tag]
    F32 = mybir.dt.float32
    GELU = {
        "tanh": mybir.ActivationFunctionType.Gelu_apprx_tanh,
        "erf": mybir.ActivationFunctionType.Gelu,
    }[gelu_tag]
    mm_cast = (lambda ap: ap.bitcast(mybir.dt.float32r)) if dtype_tag == "f32" \
        else (lambda ap: ap)

    KH, KF = H // 128, F // 128        # 8 h-tiles, 16 f-tiles
    # Chunk layout: a 512-wide head (paces w1 streaming at ~150 GB/s during
    # the first pass) + the remainder split evenly. Every chunk must be
    # >= ~130 columns: narrower matmuls cannot hide their 128-col LDWEIGHTS
    # (~53 ns with FWL), which floors the cadence regardless of width.
    # Phase A chunks: 512-wide head (so the chunk0 pass lasts >= the ~23us
    # the w1 stream needs at ring bandwidth) + even remainder, all >= ~130
    # columns so LDWEIGHTS (~53ns/tile, unhideable below 128 cols) stays
    # covered by the matmul stream.
    # Phase B chunks: big-to-small with a 128-wide tail -- a 128-wide matmul
    # exactly matches the LDWEIGHTS floor (zero waste) and makes the final
    # copy+DMA drain tiny. gT lives in SBUF, so B may chunk independently.
    chunks_a = _chunks_a(C)
    if C <= 640:
        chunks_b = chunks_a
    elif C <= 1024:
        chunks_b = [(0, 512), (512, C - 512)]
    else:
        chunks_b = [(0, 512), (512, C - 640), (C - 128, 128)]
    chunks = chunks_a

    nc = bass.Bass()
    # x arrives pre-transposed (host transpose is free; transpose-DMA from
    # token-major was ~2x slower AND corrupted repeat executions). It is
    # split into a chunk0 tensor and a rest tensor so every DMA slice is
    # FULLY contiguous in DRAM (row stride == row length) -- lets the DMA
    # concatenate M2S reads instead of 1 KB-per-descriptor fetches on the
    # head's critical path.
    cw0 = chunks[0][1]
    xT0_d = nc.dram_tensor("xT0", [H, cw0], DT, kind="ExternalInput")
    xTr_d = nc.dram_tensor("xTr", [H, C - cw0], DT, kind="ExternalInput") \
        if C > cw0 else None
    # w1 arrives pre-tiled: w1t[fm, p, hk*128+c] = w1[hk*128+p, fm*128+c], so
    # each [128, 128] lhsT tile is a contiguous free-dim slice and one fm's
    # worth (0.5 MB) is a single DMA -- keeps the first matmul group's DMA
    # prefix tiny instead of needing all of w1.
    w1_d = nc.dram_tensor("w1t", [KF, 128, KH * 128], DT, kind="ExternalInput")
    w2_d = nc.dram_tensor("w2", [F, H], DT, kind="ExternalInput")
    # y is produced transposed ([H, C]): phase B keeps tokens on the moving
    # axis. bf16 output halves the out-DMA bytes; the host combines in fp32.
    y_d = nc.dram_tensor("y", [H, C], DT, kind="ExternalOutput")

    with tile.TileContext(nc) as tc:
        with tc.tile_pool(name="persist", bufs=1) as wp, \
             tc.tile_pool(name="psumA", bufs=5, space="PSUM") as pp_a, \
             tc.tile_pool(name="psumB", bufs=3, space="PSUM") as pp_b, \
             tc.tile_pool(name="outs", bufs=4) as op:

            xT_s = [wp.tile([128, C], DT, name=f"xT{k}") for k in range(KH)]
            w1_s = [wp.tile([128, KH * 128], DT, name=f"w1t{k}")
                    for k in range(KF)]
            w2_s = [wp.tile([128, H], DT, name=f"w2{k}") for k in range(KF)]

            # DMA issue order tracks first use. Phase A runs chunk-outer, so
            # the critical prefix is w1[0] (256 KB, the first group's weights)
            # + xT chunk0 (1 MB); later groups only need one 256 KB w1 tile
            # each ~2us. Only SP + ACT can push HWDGE.
            def xchunk(k, c0, cw, e):
                if c0 < cw0:
                    src = xT0_d[k * 128:(k + 1) * 128, c0:c0 + cw]
                else:
                    src = xTr_d[k * 128:(k + 1) * 128, c0 - cw0:c0 - cw0 + cw]
                e.dma_start(xT_s[k][:, c0:c0 + cw], src)

            # Critical-prefix DMAs only: xT chunk0 + w1[0..2]. All other
            # input DMAs are paced behind phase-A progress (below).
            # w1[0] gates the very first matmul; split it across the two
            # rings BY PARTITION so each half keeps full 2 KB descriptors.
            (c0a, cwa) = chunks[0]
            # Scalar ring: w1[0]h1, xT0, w1[1], xT1, xT2; sync ring:
            # w1[0]h2, xT3-7, w1[2]. w1[1] rides between the first xT tiles
            # so group 1 never waits on it. Offloading w1[1/2] to the GpSimd
            # SWDGE ring: +3.5us, do not resurrect.
            nc.scalar.dma_start(w1_s[0][0:64, :], w1_d[0][0:64, :])
            nc.sync.dma_start(w1_s[0][64:128, :], w1_d[0][64:128, :])
            xchunk(0, c0a, cwa, nc.scalar)
            nc.scalar.dma_start(w1_s[1][:], w1_d[1])
            xchunk(1, c0a, cwa, nc.scalar)
            xchunk(2, c0a, cwa, nc.scalar)
            for k in range(3, KH):
                xchunk(k, c0a, cwa, nc.sync)
            nc.sync.dma_start(w1_s[2][:], w1_d[2])

            # PE warm-up: the critical prefix (w1[0] + xT chunk0, ~1.25 MB)
            # cannot land before ~5us after the rings start, so the first
            # real matmul is DMA-gated no matter what. Dummy matmuls soak
            # that window: they keep HAM's activity window busy so the clock
            # flips 1.2->2.4 GHz before the real stream begins (real work
            # then runs entirely warm). 12 x 427ns cold covers the prefix.
            dmy_l = wp.tile([128, 128], DT, name="dmy_l")
            dmy_r = wp.tile([128, 512], DT, name="dmy_r")
            nc.vector.memset(dmy_l[:], 0)
            nc.vector.memset(dmy_r[:], 0)
            ps_w = pp_b.tile([128, 512], F32, name="psW", tag="psB")
            for r in range(12):
                nc.tensor.matmul(ps_w[:], mm_cast(dmy_l[:]), mm_cast(dmy_r[:]),
                                 start=(r == 0), stop=(r == 11))
            gT_s = [wp.tile([128, C], DT, name=f"gT{k}") for k in range(KF)]

            # Non-critical input DMAs, paced behind phase-A groups so the
            # transfers trail compute instead of competing with the critical
            # prefix for aggregate HBM bandwidth. pace_plan[g] = DMAs to
            # issue once group g's matmuls retire (~1.7us per warm group).
            pace_plan = {}

            def plan(g, dst, src, tr=False):
                pace_plan.setdefault(g, []).append((dst, src, tr))

            for k in range(3, KF):
                plan(max(0, k - 6), w1_s[k][:], w1_d[k])   # needed at group k
            for ci, (c0, cw) in enumerate(chunks[1:]):     # needed at chunk 1+
                for k in range(KH):
                    plan(4 + 5 * ci + k // 2, xT_s[k][:, c0:c0 + cw],
                         xTr_d[k * 128:(k + 1) * 128, c0 - cw0:c0 - cw0 + cw])
            # w2 is only needed at phase B, and phase B's FIRST group already
            # contracts over all 16 fk tiles -- so w2 must finish by phase A
            # end but should stay off the rings during the chunk0 pass, where
            # w1 streaming already saturates them (~150 GB/s consumption).
            for k in range(KF):
                plan(14 + 2 * k, w2_s[k][:], w2_d[k * 128:(k + 1) * 128, :])

            from concourse.tile_rust import add_dep_helper

            # Phase A: g^T[f, c] = gelu(sum_h w1[h, f] * x[c, h]), [F, C]
            # layout. chunk-outer: early groups reuse xT chunk0 and need just
            # one fresh w1 tile each, keeping the DMA critical path short.
            gi = 0
            for (c0, cw) in chunks:
                for fm in range(KF):
                    ps = pp_a.tile([128, 512], F32, name="psA", tag="psA")
                    for hk in range(KH):
                        mm = nc.tensor.matmul(
                            ps[:, :cw],
                            mm_cast(w1_s[fm][:, hk * 128:(hk + 1) * 128]),
                            mm_cast(xT_s[hk][:, c0:c0 + cw]),
                            start=(hk == 0),
                            stop=(hk == KH - 1),
                        )
                    nc.scalar.activation(gT_s[fm][:, c0:c0 + cw], ps[:, :cw], GELU)
                    for (dst, src, tr) in pace_plan.get(gi, []):
                        d = nc.sync.dma_start(dst, src, transpose=tr)
                        add_dep_helper(d.ins, mm.ins, sync=True,
                                       reason="dma pacing")
                    gi += 1

            # Scheduler fence between phases: without it Tile reorders the ACT
            # stream and synthesizes same-engine waits on the phase-B copies,
            # overflowing the AC struct's 1-wait budget.
            tc.no_sync_barrier()

            # Phase B: y^T[h, c] = sum_f w2[f, h] * g^T[f, c].
            # chunk-outer with the narrow chunk LAST: out-DMAs trickle behind
            # compute and the final drain is only the narrow chunk's 8 small
            # tiles instead of a full 1.2 MB hm-row. Routing-prob scaling
            # happens on the host during scatter-add.
            for ci, (c0, cw) in enumerate(chunks_b):
                last_chunk = ci == len(chunks_b) - 1
                for hm in range(KH):
                    ps = pp_b.tile([128, 512], F32, name="psB", tag="psB")
                    for fk in range(KF):
                        nc.tensor.matmul(
                            ps[:, :cw],
                            mm_cast(w2_s[fk][:, hm * 128:(hm + 1) * 128]),
                            mm_cast(gT_s[fk][:, c0:c0 + cw]),
                            start=(fk == 0),
                            stop=(fk == KF - 1),
                        )
                    # unique slot per output tile: a reused slot would add a
                    # WAR wait on the DMA-out to the copy instruction.
                    yt = op.tile([128, 512], DT, name="yt", tag="yt",
                                 bufs=KH * len(chunks_b))
                    nc.scalar.activation(
                        yt[:, :cw], ps[:, :cw],
                        mybir.ActivationFunctionType.Copy)
                    # Triggers on SP (idle in phase B; a trigger costs ~620ns
                    # of issuing-engine time). Last chunk: tiles complete
                    # every ~460ns, faster than one ring can trigger, so
                    # alternate with ACT to keep the final DMA tight behind
                    # its copy.
                    deng = (nc.scalar if hm % 2 else nc.sync) if last_chunk \
                        else nc.sync
                    deng.dma_start(
                        y_d[hm * 128:(hm + 1) * 128, c0:c0 + cw],
                        yt[:, :cw])

    _strip_redundant_same_engine_waits(nc)
    _split_overloaded_waits(nc)
    _trim_redundant_barriers(nc)
    return nc


def _make_in_maps(hidden_states, w1, w2, tok_per_e, prob_per_e, C,
                  dtype_tag="bf16"):
    np_dt = {"bf16": ml_dtypes.bfloat16, "f32": np.float32}[dtype_tag]
    cw0 = _chunks_a(C)[0][1]
    in_maps = []
    for e in range(E):
        tok = tok_per_e[e]
        xg = np.zeros((H, C), dtype=np_dt)
        xg[:, :len(tok)] = hidden_states[tok].T.astype(np_dt)
        KH, KF = H // 128, F // 128
        w1t = (w1[e].astype(np_dt).reshape(KH, 128, KF, 128)
               .transpose(2, 1, 0, 3).reshape(KF, 128, KH * 128))
        m = {
            "xT0": np.ascontiguousarray(xg[:, :cw0]),
            "w1t": np.ascontiguousarray(w1t),
            "w2": np.ascontiguousarray(w2[e].astype(np_dt)),
        }
        if C > cw0:
            m["xTr"] = np.ascontiguousarray(xg[:, cw0:])
        in_maps.append(m)
    return in_maps


def kernel(hidden_states, router_weight, w1, w2):
    from concourse.bass_utils import run_bass_kernel_spmd

    hidden_states = np.asarray(hidden_states, dtype=np.float32)
    router_weight = np.asarray(router_weight, dtype=np.float32)
    w1 = np.asarray(w1, dtype=np.float32)
    w2 = np.asarray(w2, dtype=np.float32)

    tok_per_e, prob_per_e = _route(hidden_states, router_weight)
    maxc = max(len(t) for t in tok_per_e)
    C = max(128, -(-maxc // 4) * 4)              # exact capacity, mult of 4

    dtype_tag, gelu_tag = "bf16", "tanh"
    key = (C, maxc, dtype_tag, gelu_tag)
    if key not in _BUILT:
        _BUILT[key] = _build(C, dtype_tag, gelu_tag, valid_c=maxc)
    nc = _BUILT[key]

    in_maps = _make_in_maps(hidden_states, w1, w2, tok_per_e, prob_per_e, C,
                            dtype_tag)

    res = run_bass_kernel_spmd(nc, in_maps, core_ids=list(range(E)))

    out = np.zeros((T, H), dtype=np.float32)
    for e in range(E):
        tok = tok_per_e[e]
        yT = res.results[e]["y"]          # [H, C] bf16, pad columns junk
        out[tok] += prob_per_e[e][:, None] * \
            yT[:, :len(tok)].T.astype(np.float32)
    return out



# revision 54
# speedup vs baseline: 1.0209x; 1.0088x over previous
"""MoE layer (T=4096, H=1024, F=2048, E=8, top-2) on 8 Trainium2 NeuronCores.

Strategy (expert-parallel, per the sharding hint):
  - Router runs on host (67 MFLOP, 0.02% of total work) to produce the
    token->expert dispatch; this implements the "all-to-all dispatch by
    routed expert" as host-side sharding, which is where sharding lives in
    this harness's contract (full inputs in, full output out).
  - Core e holds expert e's weights (w1[e], w2[e]) and processes only the
    tokens routed to it (capacity-padded to the max expert load).
  - Device per core: hT = w1[e]^T-oriented matmul producing g^T = gelu(x@w1)
    in [F, C] layout, then y^T = w2^T-stationary matmuls over g^T -- no
    on-device transposes needed; prob scaling happens in the host combine.
  - Host scatter-adds each expert's [count, H] slice into the [T, H] output.

Matmuls run in bf16 with fp32 PSUM accumulation (rel err ~3e-3 vs the fp32
reference on this input set; measured host-side before committing to it).
"""

import numpy as np
import ml_dtypes

T, H, F, E, TOPK = 4096, 1024, 2048, 8, 2

_BUILT = {}  # cache: (C, maxc, dtype_tag, gelu_tag) -> bass.Bass


def _chunks_a(C):
    """Phase A chunk layout: 512-wide head + even remainder, all >= ~130.
    chunk0 MUST stay 512: the w1 stream is feedback-paced off matmul-group
    retirement and tops out ~150 GB/s, which exactly matches a 512-wide
    chunk0 pass's consumption. A 364-wide chunk0 was tried (smaller prefix,
    2-ring pacing): the pass outran the w1 supply, +2.6us of stalls."""
    if C <= 640:
        return [(0, C)]
    if C <= 1024:
        return [(0, 512), (512, C - 512)]
    r = C - 512
    r1 = (r + 3) // 4 * 2
    return [(0, 512), (512, r1), (512 + r1, r - r1)]


def _route(hidden_states, router_weight):
    """Host router: returns (token_idx[e], prob[e]) per expert."""
    logits = hidden_states.astype(np.float32) @ router_weight.astype(np.float32)
    # top-2 by value (descending); ties broken like jax.lax.top_k (lowest index)
    order = np.argsort(-logits, axis=1, kind="stable")
    top_idx = order[:, :TOPK]                                   # [T, 2]
    top_vals = np.take_along_axis(logits, top_idx, axis=1)      # [T, 2]
    m = top_vals.max(axis=1, keepdims=True)
    ex = np.exp(top_vals - m)
    top_probs = ex / ex.sum(axis=1, keepdims=True)              # [T, 2]
    tok_per_e, prob_per_e = [], []
    for e in range(E):
        mask = top_idx == e                                     # [T, 2]
        tok = np.nonzero(mask.any(axis=1))[0]
        p = (top_probs * mask).sum(axis=1)[tok].astype(np.float32)
        tok_per_e.append(tok)
        prob_per_e.append(p)
    return tok_per_e, prob_per_e


def _strip_redundant_same_engine_waits(nc, margin=8):
    """Drop sem waits that FIFO program order on the same engine already
    guarantees (cumulative same-engine updates >= wait_value + margin).

    Tile occasionally synthesizes such waits (e.g. on the first phase-B copy);
    they are semantically redundant but overflow the 1-wait sync budget of the
    AC/TS instruction structs in walrus codegen. margin covers the engine
    queue depth so even in-flight-but-unretired updates can't be relied on.
    """
    from collections import defaultdict
    for f in nc.m.functions:
        counts = defaultdict(int)  # sem_id -> updates so far (program order)
        for b in f.blocks:
            for i in b.instructions:
                si = i.sync_info
                if si is None:
                    continue
                ups = [u for u in (si.on_update or [])
                       if u.sync_type == "semaphore"
                       and u.update_mode in ("sem-inc", "sem-add-imm")]
                own_sems = {u.id for u in ups}
                ws = list(si.on_wait or [])
                keep = []
                for w in ws:
                    if (w.sync_type == "semaphore"
                            and w.wait_mode == "sem-ge-imm"
                            and w.id in own_sems):
                        # wait on the stream this instruction itself belongs
                        # to (it updates the same sem): FIFO order within the
                        # stream makes it redundant once enough prior updates
                        # exist. DMA queue sems bump by 16 per transfer, so
                        # margin only applies to 1-inc engine sems.
                        m = margin if all(
                            u.update_value == 1 for u in ups if u.id == w.id
                        ) else 0
                        if counts[w.id] >= w.wait_value + m:
                            continue
                    keep.append(w)
                if len(keep) != len(ws):
                    si.on_wait = keep
                    i.sync_info = si
                for u in ups:
                    counts[u.id] += u.update_value


def _split_overloaded_waits(nc, max_waits=1):
    """The TPB instruction structs encode at most one sem-ge wait (plus
    updates); walrus errors with "Too many sync wait commands" beyond that.
    For any instruction still carrying several sem-ge waits after the
    redundancy strip, move the excess onto NOPs inserted just before it on
    the same engine -- a strictly more conservative ordering."""
    import concourse.mybir as mybir
    import bass_rust
    for f in nc.m.functions:
        for b in f.blocks:
            il = b.instructions
            idx = 0
            while idx < len(il):
                i = il[idx]
                si = i.sync_info
                if type(i).__name__ == "InstEventSemaphore" or si is None:
                    idx += 1
                    continue
                sem_ws = [w for w in (si.on_wait or [])
                          if w.sync_type == "semaphore"
                          and w.wait_mode == "sem-ge-imm"]
                other_ws = [w for w in (si.on_wait or []) if w not in sem_ws]
                if len(sem_ws) > max_waits:
                    si.on_wait = other_ws + sem_ws[-max_waits:]
                    i.sync_info = si
                    rest = sem_ws[:-max_waits]
                    pos = idx
                    for j in range(0, len(rest), max_waits):
                        n = mybir.InstNoOp(
                            name=nc.get_next_instruction_name(),
                            ins=[], outs=[])
                        n.engine = i.engine
                        n.sync_info = bass_rust.SyncInfo(
                            on_wait=rest[j:j + max_waits], on_update=[])
                        il.insert(pos, n)
                        pos += 1
                        idx += 1
                idx += 1


def _trim_redundant_barriers(nc):
    """Remove the Tile entry barrier from the main block (delays the first
    DMA trigger ~0.7us; body data deps are fully covered by per-tile sems --
    the only thing it orders is const-tile memsets, which finish ~8us before
    their first reader) and the second end-block barrier round (the runtime
    epilogue's own butterfly follows immediately)."""
    bar_ids = {int(k) for k, v in nc.m.ant_sem_names.items()
               if any("barrier_" in n for n in v)}

    def is_bar(i):
        si = i.sync_info
        if si is None or type(i).__name__ not in ("InstDrain",
                                                  "InstEventSemaphore"):
            return False
        ids = [w.id for w in (si.on_wait or [])] +               [u.id for u in (si.on_update or [])]
        return bool(ids) and all(x in bar_ids for x in ids)

    f = nc.m.functions[0]
    main = f.blocks[0]
    main.instructions[:] = [i for i in main.instructions if not is_bar(i)]
    endb = f.blocks[-1]
    il = endb.instructions
    isa = [ix for ix, i in enumerate(il) if type(i).__name__ == "InstISA"]
    if isa:
        cut = isa[-1] + 1
        il[:] = il[:cut] + [i for i in il[cut:] if not is_bar(i)]


def _build(C, dtype_tag="bf16", gelu_tag="tanh", valid_c=None):
    """Build the per-core Bass program. C = token capacity (multiple of 4,
    set to the exact max expert load rounded up); every column is computed."""
    import concourse.bass as bass
    import concourse.mybir as mybir
    import concourse.tile as tile

    DT = {"bf16": mybir.dt.bfloat16, "f32": mybir.dt.float32}[dtype_tag]
    F32 = mybir.dt.float32
    GELU = {
        "tanh": mybir.ActivationFunctionType.Gelu_apprx_tanh,
        "erf": mybir.ActivationFunctionType.Gelu,
    }[gelu_tag]
    mm_cast = (lambda ap: ap.bitcast(mybir.dt.float32r)) if dtype_tag == "f32" \
        else (lambda ap: ap)

    KH, KF = H // 128, F // 128        # 8 h-tiles, 16 f-tiles
    # Chunk layout: a 512-wide head (paces w1 streaming at ~150 GB/s during
    # the first pass) + the remainder split evenly. Every chunk must be
    # >= ~130 columns: narrower matmuls cannot hide their 128-col LDWEIGHTS
    # (~53 ns with FWL), which floors the cadence regardless of width.
    # Phase A chunks: 512-wide head (so the chunk0 pass lasts >= the ~23us
    # the w1 stream needs at ring bandwidth) + even remainder, all >= ~130
    # columns so LDWEIGHTS (~53ns/tile, unhideable below 128 cols) stays
    # covered by the matmul stream.
    # Phase B chunks: big-to-small with a 128-wide tail -- a 128-wide matmul
    # exactly matches the LDWEIGHTS floor (zero waste) and makes the final
    # copy+DMA drain tiny. gT lives in SBUF, so B may chunk independently.
    chunks_a = _chunks_a(C)
    if C <= 640:
        chunks_b = chunks_a
    elif C <= 1024:
        chunks_b = [(0, 512), (512, C - 512)]
    else:
        chunks_b = [(0, 512), (512, C - 640), (C - 128, 128)]
    chunks = chunks_a

    nc = bass.Bass()
    # x arrives pre-transposed (host transpose is free; transpose-DMA from
    # token-major was ~2x slower AND corrupted repeat executions). It is
    # split into a chunk0 tensor and a rest tensor so every DMA slice is
    # FULLY contiguous in DRAM (row stride == row length) -- lets the DMA
    # concatenate M2S reads instead of 1 KB-per-descriptor fetches on the
    # head's critical path.
    cw0 = chunks[0][1]
    xT0_d = nc.dram_tensor("xT0", [H, cw0], DT, kind="ExternalInput")
    xTr_d = nc.dram_tensor("xTr", [H, C - cw0], DT, kind="ExternalInput") \
        if C > cw0 else None
    # w1 arrives pre-tiled: w1t[fm, p, hk*128+c] = w1[hk*128+p, fm*128+c], so
    # each [128, 128] lhsT tile is a contiguous free-dim slice and one fm's
    # worth (0.5 MB) is a single DMA -- keeps the first matmul group's DMA
    # prefix tiny instead of needing all of w1.
    w1_d = nc.dram_tensor("w1t", [KF, 128, KH * 128], DT, kind="ExternalInput")
    w2_d = nc.dram_tensor("w2", [F, H], DT, kind="ExternalInput")
    # y is produced transposed ([H, C]): phase B keeps tokens on the moving
    # axis. bf16 output halves the out-DMA bytes; the host combines in fp32.
    y_d = nc.dram_tensor("y", [H, C], DT, kind="ExternalOutput")

    with tile.TileContext(nc) as tc:
        with tc.tile_pool(name="persist", bufs=1) as wp, \
             tc.tile_pool(name="psumA", bufs=5, space="PSUM") as pp_a, \
             tc.tile_pool(name="psumB", bufs=3, space="PSUM") as pp_b, \
             tc.tile_pool(name="outs", bufs=4) as op:

            xT_s = [wp.tile([128, C], DT, name=f"xT{k}") for k in range(KH)]
            w1_s = [wp.tile([128, KH * 128], DT, name=f"w1t{k}")
                    for k in range(KF)]
            w2_s = [wp.tile([128, H], DT, name=f"w2{k}") for k in range(KF)]

            # DMA issue order tracks first use. Phase A runs chunk-outer, so
            # the critical prefix is w1[0] (256 KB, the first group's weights)
            # + xT chunk0 (1 MB); later groups only need one 256 KB w1 tile
            # each ~2us. Only SP + ACT can push HWDGE.
            def xchunk(k, c0, cw, e):
                if c0 < cw0:
                    src = xT0_d[k * 128:(k + 1) * 128, c0:c0 + cw]
                else:
                    src = xTr_d[k * 128:(k + 1) * 128, c0 - cw0:c0 - cw0 + cw]
                e.dma_start(xT_s[k][:, c0:c0 + cw], src)

            # Critical-prefix DMAs only: xT chunk0 + w1[0..2]. All other
            # input DMAs are paced behind phase-A progress (below).
            # w1[0] gates the very first matmul; split it across the two
            # rings BY PARTITION so each half keeps full 2 KB descriptors.
            (c0a, cwa) = chunks[0]
            # Scalar ring: w1[0]h1, xT0, w1[1], xT1, xT2; sync ring:
            # w1[0]h2, xT3-7, w1[2]. w1[1] rides between the first xT tiles
            # so group 1 never waits on it. Offloading w1[1/2] to the GpSimd
            # SWDGE ring: +3.5us, do not resurrect.
            nc.scalar.dma_start(w1_s[0][0:64, :], w1_d[0][0:64, :])
            nc.sync.dma_start(w1_s[0][64:128, :], w1_d[0][64:128, :])
            xchunk(0, c0a, cwa, nc.scalar)
            nc.scalar.dma_start(w1_s[1][:], w1_d[1])
            xchunk(1, c0a, cwa, nc.scalar)
            xchunk(2, c0a, cwa, nc.scalar)
            for k in range(3, KH):
                xchunk(k, c0a, cwa, nc.sync)
            nc.sync.dma_start(w1_s[2][:], w1_d[2])

            # PE warm-up: the critical prefix (w1[0] + xT chunk0, ~1.25 MB)
            # cannot land before ~5us after the rings start, so the first
            # real matmul is DMA-gated no matter what. Dummy matmuls soak
            # that window: they keep HAM's activity window busy so the clock
            # flips 1.2->2.4 GHz before the real stream begins (real work
            # then runs entirely warm). 12 x 427ns cold covers the prefix.
            dmy_l = wp.tile([128, 128], DT, name="dmy_l")
            dmy_r = wp.tile([128, 512], DT, name="dmy_r")
            nc.vector.memset(dmy_l[:], 0)
            nc.vector.memset(dmy_r[:], 0)
            ps_w = pp_b.tile([128, 512], F32, name="psW", tag="psB")
            for r in range(13):
                nc.tensor.matmul(ps_w[:], mm_cast(dmy_l[:]), mm_cast(dmy_r[:]),
                                 start=(r == 0), stop=(r == 12))
            gT_s = [wp.tile([128, C], DT, name=f"gT{k}") for k in range(KF)]

            # Non-critical input DMAs, paced behind phase-A groups so the
            # transfers trail compute instead of competing with the critical
            # prefix for aggregate HBM bandwidth. pace_plan[g] = DMAs to
            # issue once group g's matmuls retire (~1.7us per warm group).
            pace_plan = {}

            def plan(g, dst, src, tr=False):
                pace_plan.setdefault(g, []).append((dst, src, tr))

            for k in range(3, KF):
                plan(max(0, k - 6), w1_s[k][:], w1_d[k])   # needed at group k
            for ci, (c0, cw) in enumerate(chunks[1:]):     # needed at chunk 1+
                for k in range(KH):
                    plan(4 + 5 * ci + k // 2, xT_s[k][:, c0:c0 + cw],
                         xTr_d[k * 128:(k + 1) * 128, c0 - cw0:c0 - cw0 + cw])
            # w2 is only needed at phase B, and phase B's FIRST group already
            # contracts over all 16 fk tiles -- so w2 must finish by phase A
            # end but should stay off the rings during the chunk0 pass, where
            # w1 streaming already saturates them (~150 GB/s consumption).
            for k in range(KF):
                plan(14 + 2 * k, w2_s[k][:], w2_d[k * 128:(k + 1) * 128, :])

            from concourse.tile_rust import add_dep_helper

            # Phase A: g^T[f, c] = gelu(sum_h w1[h, f] * x[c, h]), [F, C]
            # layout. chunk-outer: early groups reuse xT chunk0 and need just
            # one fresh w1 tile each, keeping the DMA critical path short.
            gi = 0
            for (c0, cw) in chunks:
                for fm in range(KF):
                    ps = pp_a.tile([128, 512], F32, name="psA", tag="psA")
                    for hk in range(KH):
                        mm = nc.tensor.matmul(
                            ps[:, :cw],
                            mm_cast(w1_s[fm][:, hk * 128:(hk + 1) * 128]),
                            mm_cast(xT_s[hk][:, c0:c0 + cw]),
                            start=(hk == 0),
                            stop=(hk == KH - 1),
                        )
                    nc.scalar.activation(gT_s[fm][:, c0:c0 + cw], ps[:, :cw], GELU)
                    for (dst, src, tr) in pace_plan.get(gi, []):
                        d = nc.sync.dma_start(dst, src, transpose=tr)
                        add_dep_helper(d.ins, mm.ins, sync=True,
                                       reason="dma pacing")
                    gi += 1

            # Scheduler fence between phases: without it Tile reorders the ACT
            # stream and synthesizes same-engine waits on the phase-B copies,
            # overflowing the AC struct's 1-wait budget.
            tc.no_sync_barrier()

            # Phase B: y^T[h, c] = sum_f w2[f, h] * g^T[f, c].
            # chunk-outer with the narrow chunk LAST: out-DMAs trickle behind
            # compute and the final drain is only the narrow chunk's 8 small
            # tiles instead of a full 1.2 MB hm-row. Routing-prob scaling
            # happens on the host during scatter-add.
            for ci, (c0, cw) in enumerate(chunks_b):
                last_chunk = ci == len(chunks_b) - 1
                for hm in range(KH):
                    ps = pp_b.tile([128, 512], F32, name="psB", tag="psB")
                    for fk in range(KF):
                        nc.tensor.matmul(
                            ps[:, :cw],
                            mm_cast(w2_s[fk][:, hm * 128:(hm + 1) * 128]),
                            mm_cast(gT_s[fk][:, c0:c0 + cw]),
                            start=(fk == 0),
                            stop=(fk == KF - 1),
                        )
                    # unique slot per output tile: a reused slot would add a
                    # WAR wait on the DMA-out to the copy instruction.
                    yt = op.tile([128, 512], DT, name="yt", tag="yt",
                                 bufs=KH * len(chunks_b))
                    nc.scalar.activation(
                        yt[:, :cw], ps[:, :cw],
                        mybir.ActivationFunctionType.Copy)
                    # Triggers on SP (idle in phase B; a trigger costs ~620ns
                    # of issuing-engine time). Last chunk: tiles complete
                    # every ~460ns, faster than one ring can trigger, so
                    # alternate with ACT to keep the final DMA tight behind
                    # its copy.
                    deng = (nc.scalar if hm % 2 else nc.sync) if last_chunk \
                        else nc.sync
                    deng.dma_start(
                        y_d[hm * 128:(hm + 1) * 128, c0:c0 + cw],
                        yt[:, :cw])

    _strip_redundant_same_engine_waits(nc)
    _split_overloaded_waits(nc)
    _trim_redundant_barriers(nc)
    return nc


def _make_in_maps(hidden_states, w1, w2, tok_per_e, prob_per_e, C,
                  dtype_tag="bf16"):
    np_dt = {"bf16": ml_dtypes.bfloat16, "f32": np.float32}[dtype_tag]
    cw0 = _chunks_a(C)[0][1]
    in_maps = []
    for e in range(E):
        tok = tok_per_e[e]
        xg = np.zeros((H, C), dtype=np_dt)
        xg[:, :len(tok)] = hidden_states[tok].T.astype(np_dt)
        KH, KF = H // 128, F // 128
        w1t = (w1[e].astype(np_dt).reshape(KH, 128, KF, 128)
               .transpose(2, 1, 0, 3).reshape(KF, 128, KH * 128))
        m = {
            "xT0": np.ascontiguousarray(xg[:, :cw0]),
            "w1t": np.ascontiguousarray(w1t),
            "w2": np.ascontiguousarray(w2[e].astype(np_dt)),
        }
        if C > cw0:
            m["xTr"] = np.ascontiguousarray(xg[:, cw0:])
        in_maps.append(m)
    return in_maps


def kernel(hidden_states, router_weight, w1, w2):
    from concourse.bass_utils import run_bass_kernel_spmd

    hidden_states = np.asarray(hidden_states, dtype=np.float32)
    router_weight = np.asarray(router_weight, dtype=np.float32)
    w1 = np.asarray(w1, dtype=np.float32)
    w2 = np.asarray(w2, dtype=np.float32)

    tok_per_e, prob_per_e = _route(hidden_states, router_weight)
    maxc = max(len(t) for t in tok_per_e)
    C = max(128, -(-maxc // 4) * 4)              # exact capacity, mult of 4

    dtype_tag, gelu_tag = "bf16", "tanh"
    key = (C, maxc, dtype_tag, gelu_tag)
    if key not in _BUILT:
        _BUILT[key] = _build(C, dtype_tag, gelu_tag, valid_c=maxc)
    nc = _BUILT[key]

    in_maps = _make_in_maps(hidden_states, w1, w2, tok_per_e, prob_per_e, C,
                            dtype_tag)

    res = run_bass_kernel_spmd(nc, in_maps, core_ids=list(range(E)))

    out = np.zeros((T, H), dtype=np.float32)
    for e in range(E):
        tok = tok_per_e[e]
        yT = res.results[e]["y"]          # [H, C] bf16, pad columns junk
        out[tok] += prob_per_e[e][:, None] * \
            yT[:, :len(tok)].T.astype(np.float32)
    return out

